# revision 37
# baseline (speedup 1.0000x reference)
"""Grouped Conv1d (B=4, T=512, G=129, F=96 -> O=96, K=3, pad=1) on 8 trn2 cores.

Sharding: 129 groups = 16 full groups per core + group 128 split across all
8 cores by (batch b = core//2, T-half = core%2).  SPMD: every core runs the
identical program on its own slice.
"""

from contextlib import ExitStack

import numpy as np

import concourse.bass as bass
import concourse.mybir as mybir
import concourse.tile as tile
from concourse import bacc
from concourse.bass_utils import run_bass_kernel_spmd

B, T, G, F, O, K = 4, 512, 129, 96, 96, 3
NCORES = 8
GPC = 16
NG = GPC + 1
TP = T + 2
TE = T // 2
TEP = TE + 2
GB = 2
NB = GPC // GB


def build_program():
    nc = bacc.Bacc("TRN2", target_bir_lowering=False, debug=False,
                   num_devices=NCORES)

    f32 = mybir.dt.float32
    f16 = mybir.dt.float16

    xm = nc.dram_tensor("xm", [NB, F, GB, B, TP], f16, kind="ExternalInput")
    xe = nc.dram_tensor("xe", [F, TEP], f16, kind="ExternalInput")
    wt = nc.dram_tensor("wt", [F, NG * K * O], f16, kind="ExternalInput")
    bt = nc.dram_tensor("bt", [O, NG], f32, kind="ExternalInput")
    om = nc.dram_tensor("om", [NB, O, GB, B, T], f16, kind="ExternalOutput")
    oe = nc.dram_tensor("oe", [O, TE], f16, kind="ExternalOutput")

    with ExitStack() as ctx:
        tc = ctx.enter_context(tile.TileContext(nc))
        wpool = ctx.enter_context(tc.tile_pool(name="w", bufs=1))
        xpool = ctx.enter_context(tc.tile_pool(name="x", bufs=5))
        opool = ctx.enter_context(tc.tile_pool(name="o", bufs=3))
        pspool = ctx.enter_context(tc.tile_pool(name="ps", bufs=8, space="PSUM"))

        w_sb = wpool.tile([F, NG * K * O], f16)
        b_sb = wpool.tile([O, NG], f32)
        xe_sb = wpool.tile([F, TEP], f16)

        x_tiles = {}

        def load_x(ib, split=False):
            x_sb = xpool.tile([F, GB * B * TP], f16, tag="x", name=f"x{ib}")
            x_tiles[ib] = x_sb
            if not split:
                h = GB * B * TP // 2
                e0 = nc.scalar if ib % 2 == 0 else nc.sync
                e1 = nc.sync if ib % 2 == 0 else nc.scalar
                src = xm[ib].rearrange("f g b t -> f (g b t)")
                e0.dma_start(x_sb[:, :h], src[:, :h])
                e1.dma_start(x_sb[:, h:], src[:, h:])

        def x_piece(ib, u0, u1, eng):
            eng.dma_start(
                x_tiles[ib][:, u0 * TP:u1 * TP],
                xm[ib].rearrange("f g b t -> f (g b t)")[:, u0 * TP:u1 * TP])

        kw = K * O
        load_x(0, split=True)
        load_x(1, split=True)
        nc.sync.dma_start(w_sb[:, :2 * kw], wt[:, :2 * kw])
        x_piece(0, 0, 1, nc.scalar)
        x_piece(0, 4, 6, nc.gpsimd)
        x_piece(0, 1, 2, nc.sync)
        x_piece(0, 2, 4, nc.scalar)
        nc.scalar.dma_start(w_sb[:, 2 * kw:8 * kw],
                            wt[:, 2 * kw:8 * kw])
        x_piece(0, 6, 8, nc.gpsimd)
        nc.scalar.dma_start(b_sb[:], bt[:])
        nc.sync.dma_start(xe_sb[:], xe[:])
        nc.sync.dma_start(w_sb[:, 8 * kw:], wt[:, 8 * kw:])
        x_piece(1, 4, 8, nc.gpsimd)
        x_piece(1, 0, 2, nc.scalar)
        x_piece(1, 2, 4, nc.sync)

        for ib in range(NB):
            if ib + 2 < NB:
                load_x(ib + 2)
            x_sb = x_tiles.pop(ib)
            o_sb = opool.tile([O, GB * B * T], f16, tag="o")
            for j in range(GB):
                i = ib * GB + j
                pss = [pspool.tile([O, T], f32, tag="ps", name=f"ps{b}")
                       for b in range(B)]
                for k in range(K):
                    for b in range(B):
                        nc.tensor.matmul(
                            pss[b][:],
                            w_sb[:, (i * K + k) * O:(i * K + k + 1) * O],
                            x_sb[:, (j * B + b) * TP + k:(j * B + b) * TP + k + T],
                            start=(k == 0),
                            stop=(k == K - 1),
                        )
                for b in range(B):
                    dst = o_sb[:, (j * B + b) * T:(j * B + b + 1) * T]
                    if (j * B + b) % 2 == 0:
                        nc.scalar.add(dst, pss[b][:], b_sb[:, i:i + 1])
                    else:
                        nc.vector.tensor_scalar_add(dst, pss[b][:],
                                                    b_sb[:, i:i + 1])
                if ib == NB - 1:
                    # last tile: store each group's output right after its
                    # copies (2048B-line halves on both rings) so the first
                    # group's 393KB drains ~2.5us earlier and only the
                    # second group's output remains after the last copies
                    om_flat = om[ib].rearrange("o g b t -> o (g b t)")
                    hj = B * T // 2
                    for p in range(2):
                        eng = (nc.sync, nc.scalar)[(2 * j + p) % 2]
                        c0 = j * B * T + p * hj
                        eng.dma_start(om_flat[:, c0:c0 + hj],
                                      o_sb[:, c0:c0 + hj])
            if ib < NB - 1:
                om_flat = om[ib].rearrange("o g b t -> o (g b t)")
                st0 = nc.sync if ib % 2 == 0 else nc.scalar
                st1 = nc.scalar if ib % 2 == 0 else nc.sync
                op = GB * B * T // 2
                for p in range(2):
                    eng = st0 if p % 2 == 0 else st1
                    eng.dma_start(om_flat[:, p * op:(p + 1) * op],
                                  o_sb[:, p * op:(p + 1) * op])

        # tail group (g=128) computed LAST: its 49KB store is the final
        # drain item instead of the last big group's 393KB, so the big
        # stores overlap the tail matmuls and the end chain is one short
        # copy + two 25KB stores on both rings
        ps = pspool.tile([O, TE], f32, tag="ps")
        for k in range(K):
            nc.tensor.matmul(
                ps[:],
                w_sb[:, (GPC * K + k) * O:(GPC * K + k + 1) * O],
                xe_sb[:, k:k + TE],
                start=(k == 0),
                stop=(k == K - 1),
            )
        oe_sb = wpool.tile([O, TE], f16)
        he = TE // 2
        nc.scalar.add(oe_sb[:, :he], ps[:, :he], b_sb[:, GPC:GPC + 1])
        nc.vector.tensor_scalar_add(oe_sb[:, he:], ps[:, he:],
                                    b_sb[:, GPC:GPC + 1])
        nc.sync.dma_start(oe[:, :he], oe_sb[:, :he])
        nc.scalar.dma_start(oe[:, he:], oe_sb[:, he:])

    nc.finalize()
    return nc


def shard_inputs(x, weight, bias):
    x = np.ascontiguousarray(x, dtype=np.float32)
    weight = np.ascontiguousarray(weight, dtype=np.float32)
    bias = np.ascontiguousarray(bias, dtype=np.float32)

    xp = np.pad(x, ((0, 0), (1, 1), (0, 0), (0, 0)))
    xt = xp.transpose(2, 3, 0, 1).astype(np.float16)
    wtr = weight.transpose(2, 0, 3, 1).astype(np.float16)

    in_maps = []
    for c in range(NCORES):
        gs = list(range(c * GPC, (c + 1) * GPC)) + [G - 1]
        b_c, t0 = c // 2, (c % 2) * TE
        xm_c = xt[c * GPC:(c + 1) * GPC].reshape(NB, GB, F, B, TP)
        in_maps.append({
            "xm": np.ascontiguousarray(xm_c.transpose(0, 2, 1, 3, 4)),
            "xe": np.ascontiguousarray(xt[G - 1, :, b_c, t0:t0 + TEP]),
            "wt": np.ascontiguousarray(wtr[:, gs].reshape(F, NG * K * O)),
            "bt": np.ascontiguousarray(bias[gs].T),
            })
    return in_maps


def unshard_outputs(results):
    out = np.empty((B, T, G, O), dtype=np.float32)
    for c in range(NCORES):
        om = results[c]["om"].astype(np.float32)
        om = om.transpose(0, 2, 1, 3, 4).reshape(GPC, O, B, T)
        out[:, :, c * GPC:(c + 1) * GPC, :] = om.transpose(2, 3, 0, 1)
        b_c, t0 = c // 2, (c % 2) * TE
        out[b_c, t0:t0 + TE, G - 1, :] = results[c]["oe"].astype(np.float32).T
    return out


def run(x, weight, bias, **run_kwargs):
    nc = build_program()
    in_maps = shard_inputs(x, weight, bias)
    res = run_bass_kernel_spmd(nc, in_maps, list(range(NCORES)), **run_kwargs)
    return unshard_outputs(res.results), res


def kernel(x, weight, bias):
    out, _ = run(x, weight, bias)
    return out


# revision 38
# speedup vs baseline: 1.1582x; 1.1582x over previous
"""Grouped Conv1d (B=4, T=512, G=129, F=96 -> O=96, K=3, pad=1) on 8 trn2 cores.

Sharding: 129 groups = 16 full groups per core + group 128 split across all
8 cores by (batch b = core//2, T-half = core%2).  SPMD: every core runs the
identical program on its own slice.
"""

from contextlib import ExitStack

import numpy as np

import concourse.bass as bass
import concourse.mybir as mybir
import concourse.tile as tile
from concourse import bacc
from concourse.bass_utils import run_bass_kernel_spmd

B, T, G, F, O, K = 4, 512, 129, 96, 96, 3
NCORES = 8
GPC = 16
NG = GPC + 1
TP = T + 2
TE = T // 2
TEP = TE + 2
GB = 2
NB = GPC // GB


def build_program():
    nc = bacc.Bacc("TRN2", target_bir_lowering=False, debug=False,
                   num_devices=NCORES)

    f32 = mybir.dt.float32
    f16 = mybir.dt.float16

    xm = nc.dram_tensor("xm", [NB, F, GB, B, TP], f16, kind="ExternalInput")
    xe = nc.dram_tensor("xe", [F, TEP], f16, kind="ExternalInput")
    wt = nc.dram_tensor("wt", [F, NG * K * O], f16, kind="ExternalInput")
    bt = nc.dram_tensor("bt", [O, NG], f32, kind="ExternalInput")
    om = nc.dram_tensor("om", [NB, O, GB, B, T], f16, kind="ExternalOutput")
    oe = nc.dram_tensor("oe", [O, TE], f16, kind="ExternalOutput")

    with ExitStack() as ctx:
        tc = ctx.enter_context(tile.TileContext(nc))
        wpool = ctx.enter_context(tc.tile_pool(name="w", bufs=1))
        xpool = ctx.enter_context(tc.tile_pool(name="x", bufs=5))
        opool = ctx.enter_context(tc.tile_pool(name="o", bufs=3))
        pspool = ctx.enter_context(tc.tile_pool(name="ps", bufs=8, space="PSUM"))

        w_sb = wpool.tile([F, NG * K * O], f16)
        b_sb = wpool.tile([O, NG], f32)
        xe_sb = wpool.tile([F, TEP], f16)

        x_tiles = {}

        def load_x(ib, split=False):
            x_sb = xpool.tile([F, GB * B * TP], f16, tag="x", name=f"x{ib}")
            x_tiles[ib] = x_sb
            if not split:
                h = GB * B * TP // 2
                e0 = nc.scalar if ib % 2 == 0 else nc.sync
                e1 = nc.sync if ib % 2 == 0 else nc.scalar
                src = xm[ib].rearrange("f g b t -> f (g b t)")
                e0.dma_start(x_sb[:, :h], src[:, :h])
                e1.dma_start(x_sb[:, h:], src[:, h:])

        def x_piece(ib, u0, u1, eng):
            eng.dma_start(
                x_tiles[ib][:, u0 * TP:u1 * TP],
                xm[ib].rearrange("f g b t -> f (g b t)")[:, u0 * TP:u1 * TP])

        kw = K * O
        load_x(0, split=True)
        load_x(1, split=True)
        nc.sync.dma_start(w_sb[:, :2 * kw], wt[:, :2 * kw])
        x_piece(0, 0, 1, nc.scalar)
        x_piece(0, 4, 6, nc.gpsimd)
        x_piece(0, 1, 2, nc.sync)
        x_piece(0, 2, 4, nc.scalar)
        nc.scalar.dma_start(w_sb[:, 2 * kw:8 * kw],
                            wt[:, 2 * kw:8 * kw])
        x_piece(0, 6, 8, nc.gpsimd)
        nc.scalar.dma_start(b_sb[:], bt[:])
        nc.sync.dma_start(xe_sb[:], xe[:])
        nc.sync.dma_start(w_sb[:, 8 * kw:], wt[:, 8 * kw:])
        x_piece(1, 4, 8, nc.gpsimd)
        x_piece(1, 0, 2, nc.scalar)
        x_piece(1, 2, 4, nc.sync)

        for ib in range(NB):
            if ib + 2 < NB:
                load_x(ib + 2)
            x_sb = x_tiles.pop(ib)
            o_sb = opool.tile([O, GB * B * T], f16, tag="o")
            for j in range(GB):
                i = ib * GB + j
                pss = [pspool.tile([O, T], f32, tag="ps", name=f"ps{b}")
                       for b in range(B)]
                for k in range(K):
                    for b in range(B):
                        nc.tensor.matmul(
                            pss[b][:],
                            w_sb[:, (i * K + k) * O:(i * K + k + 1) * O],
                            x_sb[:, (j * B + b) * TP + k:(j * B + b) * TP + k + T],
                            start=(k == 0),
                            stop=(k == K - 1),
                        )
                for b in range(B):
                    dst = o_sb[:, (j * B + b) * T:(j * B + b + 1) * T]
                    if (j * B + b) % 2 == 0:
                        nc.scalar.add(dst, pss[b][:], b_sb[:, i:i + 1])
                    else:
                        nc.vector.tensor_scalar_add(dst, pss[b][:],
                                                    b_sb[:, i:i + 1])
            om_flat = om[ib].rearrange("o g b t -> o (g b t)")
            st0 = nc.sync if ib % 2 == 0 else nc.scalar
            st1 = nc.scalar if ib % 2 == 0 else nc.sync
            parts = 4 if ib == NB - 1 else 2
            op = GB * B * T // parts
            for p in range(parts):
                eng = st0 if p % 2 == 0 else st1
                eng.dma_start(om_flat[:, p * op:(p + 1) * op],
                              o_sb[:, p * op:(p + 1) * op])

            if ib == 1:
                ps = pspool.tile([O, TE], f32, tag="ps")
                for k in range(K):
                    nc.tensor.matmul(
                        ps[:],
                        w_sb[:, (GPC * K + k) * O:(GPC * K + k + 1) * O],
                        xe_sb[:, k:k + TE],
                        start=(k == 0),
                        stop=(k == K - 1),
                    )
                oe_sb = wpool.tile([O, TE], f16)
                nc.vector.tensor_scalar_add(oe_sb[:], ps[:],
                                            b_sb[:, GPC:GPC + 1])
                nc.sync.dma_start(oe[:], oe_sb[:])

    nc.finalize()
    return nc


def shard_inputs(x, weight, bias):
    x = np.ascontiguousarray(x, dtype=np.float32)
    weight = np.ascontiguousarray(weight, dtype=np.float32)
    bias = np.ascontiguousarray(bias, dtype=np.float32)

    xp = np.pad(x, ((0, 0), (1, 1), (0, 0), (0, 0)))
    xt = xp.transpose(2, 3, 0, 1).astype(np.float16)
    wtr = weight.transpose(2, 0, 3, 1).astype(np.float16)

    in_maps = []
    for c in range(NCORES):
        gs = list(range(c * GPC, (c + 1) * GPC)) + [G - 1]
        b_c, t0 = c // 2, (c % 2) * TE
        xm_c = xt[c * GPC:(c + 1) * GPC].reshape(NB, GB, F, B, TP)
        in_maps.append({
            "xm": np.ascontiguousarray(xm_c.transpose(0, 2, 1, 3, 4)),
            "xe": np.ascontiguousarray(xt[G - 1, :, b_c, t0:t0 + TEP]),
            "wt": np.ascontiguousarray(wtr[:, gs].reshape(F, NG * K * O)),
            "bt": np.ascontiguousarray(bias[gs].T),
            })
    return in_maps


def unshard_outputs(results):
    out = np.empty((B, T, G, O), dtype=np.float32)
    for c in range(NCORES):
        om = results[c]["om"].astype(np.float32)
        om = om.transpose(0, 2, 1, 3, 4).reshape(GPC, O, B, T)
        out[:, :, c * GPC:(c + 1) * GPC, :] = om.transpose(2, 3, 0, 1)
        b_c, t0 = c // 2, (c % 2) * TE
        out[b_c, t0:t0 + TE, G - 1, :] = results[c]["oe"].astype(np.float32).T
    return out


def run(x, weight, bias, **run_kwargs):
    nc = build_program()
    in_maps = shard_inputs(x, weight, bias)
    res = run_bass_kernel_spmd(nc, in_maps, list(range(NCORES)), **run_kwargs)
    return unshard_outputs(res.results), res


def kernel(x, weight, bias):
    out, _ = run(x, weight, bias)
    return out
